# revision 43
# baseline (speedup 1.0000x reference)
"""Trainium2 Bass kernel for CombinedLSTMWithStatic2Hop.

Model: per-node LSTM over T timesteps + static encoder -> fusion -> 2x SAGEConv
(mean aggregation) -> linear head.

Sharding: B*N = 8000 nodes split into 1000 contiguous nodes per core (8 cores).
Each graph (2000 nodes) spans a core pair; SAGE aggregation uses pairwise
AllGather of node features between the two halves.

LSTM layout: hidden/gate dim on partitions, nodes on the free dim, 4 chunks
of 250 nodes per timestep.  ONE sigmoid ACT op per chunk covers FIVE
256-stride psum slots [i|f|o|g|c]:
  slots 0-3: gates from one DoubleRow fp8 matmul each (g preact 2x scaled
             host-side so sigmoid gives (tanh+1)/2),
  slot 4:    2*c' of the chunk TWO instances back (lag-2 rotation), so
             tanh(c') = 2*sigmoid(2c') - 1 comes out of the same ACT op
             and the ACT engine runs 4 ops/timestep with no separate tanh.
Cell arithmetic runs on DVE scalar_tensor_tensor (InstTensorScalarPtr, 4x
perf mode on packed fp16) with the state kept as S := 2c:
  P  = (Sg * 4) * Si          ; q = (Si * -2) + P   -> q = Si*(4Sg-2)
  t1 = (S * 1) * Sf           ; S' = (t1 * 1) + q   -> S' = 2c'
  c2p: Pool writes (t1+q) into the +2 instance's psum c-slot
  Tc = 2*Sc - 1               ; h' = (Tc * 1) * So  (Pool, fp8 into xh)
x images are compact 9 rows (8 features + ones bias row) at partitions 0-8
for every t; weight rows outside 0..8 of the x-part are zero, so the fused
[W_hh | W_x] DoubleRow weight is group-independent: w_f is [128, 1024] fp8.
"""

import os as _os

import ml_dtypes
import numpy as np

BFNP = np.float16

import concourse.bass as bass
import concourse.tile as tile
from concourse import bacc, mybir
from concourse.bass_utils import run_bass_kernel_spmd
from concourse.masks import make_identity

F32 = mybir.dt.float32
BF16 = mybir.dt.float16  # fp16: same PE rate as bf16, 8x finer mantissa
E4 = mybir.dt.float8e4   # fp8 e4m3: DoubleRow matmul at 2 cols/cycle
E4NP = ml_dtypes.float8_e4m3fn

B, T, N, E = 4, 96, 2000, 16000
F_DYN, F_STA, H = 8, 16, 128
N_CORES = 8
NPC = B * N // N_CORES      # 1000 nodes per core
CH = NPC // 2               # 500 node half (GNN phase)
CK = NPC // 4               # 250 node chunk (LSTM)
SL = 256                    # psum slot stride (f32 elems); 5 slots per chunk
GSL = 512                   # GNN psum slot stride (one bank)

# module-level knobs (test.py may override)
TRACE = False
TRACE_KW = {}

_PROG_CACHE = {}


def _build_program(t_steps=T, repeat=1, gnn_repeat=1):
    nc = bacc.Bacc("TRN2", target_bir_lowering=False, debug=False,
                   num_devices=N_CORES)

    # ---- DRAM I/O ----
    x8c = nc.dram_tensor("x8c", [t_steps * 9, NPC], E4, kind="ExternalInput")
    w_f = nc.dram_tensor("w_f", [128, 1024], E4, kind="ExternalInput")
    w_sta = nc.dram_tensor("w_sta", [F_STA + 1, H], F32, kind="ExternalInput")
    sta_t = nc.dram_tensor("sta_t", [F_STA + 1, NPC], F32, kind="ExternalInput")
    w_fz = nc.dram_tensor("w_fz", [H, H], BF16, kind="ExternalInput")
    w_fs = nc.dram_tensor("w_fs", [H, H], BF16, kind="ExternalInput")
    b_fu = nc.dram_tensor("b_fu", [H, 1], F32, kind="ExternalInput")
    w_r1 = nc.dram_tensor("w_r1", [H, H], BF16, kind="ExternalInput")
    w_l1 = nc.dram_tensor("w_l1", [H, H], BF16, kind="ExternalInput")
    b_l1 = nc.dram_tensor("b_l1", [H, 1], F32, kind="ExternalInput")
    w_r2 = nc.dram_tensor("w_r2", [H, H], BF16, kind="ExternalInput")
    w_l2 = nc.dram_tensor("w_l2", [H, H], BF16, kind="ExternalInput")
    b_l2 = nc.dram_tensor("b_l2", [H, 1], F32, kind="ExternalInput")
    w_ou = nc.dram_tensor("w_ou", [H, 1], BF16, kind="ExternalInput")
    b_ou = nc.dram_tensor("b_ou", [1, 1], F32, kind="ExternalInput")
    a_mat = nc.dram_tensor("a_mat", [N, N], BF16, kind="ExternalInput")
    out_d = nc.dram_tensor("out", [1, NPC], F32, kind="ExternalOutput")

    AT = mybir.AluOpType
    AF = mybir.ActivationFunctionType
    DR = mybir.MatmulPerfMode.DoubleRow

    with tile.TileContext(nc) as tc:
        with (
            tc.tile_pool(name="const", bufs=1) as cp,
            tc.tile_pool(name="xp", bufs=3) as xp,
            tc.tile_pool(name="wk", bufs=2) as wk,
        ):
            # ---- t=0 critical path first on the sync queue: fused weights
            # then the x(0) image ----
            w_f_t = cp.tile([128, 1024], E4, tag="w_f")
            nc.sync.dma_start(out=w_f_t[:, :], in_=w_f[:, :])

            # 3 persistent [h(t) | x(t+1)] buffers rotated manually (pool
            # reuse would trip the race checker on the once-only memset of
            # x-part rows 9..127, which stay zero forever)
            xh_bufs = []
            for i in range(3):
                xb = wk.tile([128, 2 * NPC], E4, tag=f"hb{i}", bufs=1,
                             name=f"xhb{i}")
                nc.vector.memset(xb[:, :] if i == 0 else xb[:, NPC:2 * NPC],
                                 0.0)
                xh_bufs.append(xb)
            xh_first = xh_bufs[0]
            nc.sync.dma_start(out=xh_first[0:9, NPC:2 * NPC],
                              in_=x8c[0:9, :])

            def cload(dram, shape, tag, dt=F32, eng=nc.gpsimd):
                tl = cp.tile(shape, dt, tag=tag)
                eng.dma_start(out=tl[:, :], in_=dram[:, :])
                return tl

            # static-encoder inputs on the (early-idle) ACT queue so the
            # static encoder fills the w_f-load gap without delaying w_f
            w_sta_tt = cload(w_sta, [F_STA + 1, H], "w_sta", eng=nc.scalar)
            sta_tt = cload(sta_t, [F_STA + 1, NPC], "sta_t", eng=nc.scalar)
            w_fz_t = cload(w_fz, [H, H], "w_fz", BF16)
            w_fs_t = cload(w_fs, [H, H], "w_fs", BF16)
            b_fu_t = cload(b_fu, [H, 1], "b_fu")
            w_r1_t = cload(w_r1, [H, H], "w_r1", BF16)
            w_l1_t = cload(w_l1, [H, H], "w_l1", BF16)
            b_l1_t = cload(b_l1, [H, 1], "b_l1")
            w_r2_t = cload(w_r2, [H, H], "w_r2", BF16)
            w_l2_t = cload(w_l2, [H, H], "w_l2", BF16)
            b_l2_t = cload(b_l2, [H, 1], "b_l2")
            w_ou_t = cload(w_ou, [H, 1], "w_ou", BF16)
            b_ou_t = cload(b_ou, [1, 1], "b_ou")

            ident = cp.tile([128, 128], BF16, tag="ident")
            make_identity(nc, ident[:, :])

            # adjacency tiles: allocated now, DMAs spread through the LSTM
            # loop on the gpsimd queue (Pool has slack between cell ops)
            a_tiles = []
            KC = N // 16  # 125-row src chunks over the FULL graph adjacency
            for k in range(16):
                a_tiles.append(cp.tile([KC, N], BF16, tag=f"a{k}",
                                       name=f"a{k}"))

            w_f_r = w_f_t[:, :].rearrange("p (s m) -> p s m", s=2)

            # ---- LSTM ----
            pl_cm = tc.tile_pool(name="psl", bufs=2, space="PSUM")
            pl = pl_cm.__enter__()

            # static encoder first: fills the w_f-load gap; its psum tile
            # aliases ps[1]'s buffer (read drained before instance idx=1)
            stl = wk.tile([128, NPC], BF16, tag="stl", bufs=1)
            pss = pl.tile([128, 5 * SL], F32, tag="gates", name="pss")
            for dc in range(4):
                nc.tensor.matmul(
                    out=pss[:, SL * dc:SL * dc + CK],
                    lhsT=w_sta_tt[0:17, :],
                    rhs=sta_tt[0:17, CK * dc:CK * dc + CK],
                    start=True, stop=True)
            nc.scalar.activation(
                out=stl[:, :].rearrange("p (c b) -> p c b", c=4),
                in_=pss[:, 0:4 * SL].rearrange("p (c b) -> p c b", c=4)
                [:, :, 0:CK],
                func=AF.Relu, scale=1.0)

            ps = [pl.tile([128, 5 * SL], F32, tag="gates", name="gates0"),
                  pl.tile([128, 5 * SL], F32, tag="gates", name="gates1")]
            # c-slot read by A(0)/A(1) before any c2p lands
            for pst in ps:
                nc.vector.memset(pst[:, 4 * SL:4 * SL + CK], 0.0)

            # final LSTM hidden state (fp16) for the fusion matmul
            hh = wk.tile([128, NPC], BF16, tag="hh16", bufs=1)

            rep_cm = tc.For_i(0, repeat, 1) if repeat > 1 else None
            if rep_cm is not None:
                rep_cm.__enter__()

            s_prev = None
            xh_prev = xh_first
            tt_tiles = {}
            for t in range(t_steps):
                s_cur = wk.tile([128, NPC], BF16, tag="s", bufs=2)
                if t + 1 < t_steps:
                    xh_cur = xh_bufs[(t + 1) % 3]
                    nc.sync.dma_start(
                        out=xh_cur[0:9, NPC:2 * NPC],
                        in_=x8c[9 * (t + 1):9 * (t + 1) + 9, :])
                else:
                    xh_cur = None
                xh_rhs = xh_prev[:, :].rearrange("p (s n) -> p s n", s=2)

                for c in range(4):
                    idx = 4 * t + c
                    pst = ps[idx % 2]
                    csl = slice(CK * c, CK * c + CK)

                    # gates: one DoubleRow mm per slot (W_hh@h + W_x@x)
                    for gi in range(4):
                        nc.tensor.matmul(
                            out=pst[:, SL * gi:SL * gi + CK],
                            lhsT=w_f_r[:, :, H * gi:H * gi + H],
                            rhs=xh_rhs[:, :, csl],
                            start=True, stop=True, perf_mode=DR)

                    # ONE sigmoid over 5 slots: [Si|Sf|So|Sg|Sc]
                    tt = wk.tile([128, 5 * CK], BF16, tag=f"T{c}",
                                 name=f"tt{c}_{t}")
                    nc.scalar.activation(
                        out=tt[:, :].rearrange("p (g b) -> p g b", g=5),
                        in_=pst[:, :].rearrange("p (s b) -> p s b", s=5)
                        [:, :, 0:CK],
                        func=AF.Sigmoid, scale=1.0)
                    si = tt[:, 0:CK]
                    sf = tt[:, CK:2 * CK]
                    sg = tt[:, 3 * CK:4 * CK]
                    sc = tt[:, 4 * CK:5 * CK]

                    # cell math, S' = Sf*S + Si*(4Sg-2), on ts(4x)/TT(2x)
                    # ops (scalar_tensor_tensor has NO dve perf modes)
                    tg_t = wk.tile([128, CK], BF16, tag=f"p{c}")
                    nc.vector.tensor_scalar(
                        out=tg_t[:, :], in0=sg, scalar1=4.0, scalar2=-2.0,
                        op0=AT.mult, op1=AT.add)
                    t2_t = wk.tile([128, CK], BF16, tag=f"q{c}")
                    nc.vector.tensor_tensor(
                        out=t2_t[:, :], in0=si, in1=tg_t[:, :], op=AT.mult)
                    if t > 0:
                        # t1 on Pool (GPSIMD cannot access PSUM, SBUF ok)
                        t1_t = wk.tile([128, CK], BF16, tag=f"u{c}")
                        nc.gpsimd.tensor_tensor(
                            out=t1_t[:, :], in0=s_prev[:, csl], in1=sf,
                            op=AT.mult)
                        nc.vector.tensor_tensor(
                            out=s_cur[:, csl], in0=t1_t[:, :],
                            in1=t2_t[:, :], op=AT.add)
                    else:
                        nc.vector.tensor_copy(out=s_cur[:, csl],
                                              in_=t2_t[:, :])
                    if not (t == t_steps - 1 and c >= 2):
                        # c-slot psum write via PE identity matmul: the
                        # only engine with cheap PSUM writes (PE is idle)
                        nc.tensor.matmul(
                            out=pst[:, 4 * SL:4 * SL + CK],
                            lhsT=ident[:, :], rhs=s_cur[:, csl],
                            start=True, stop=True)

                    # carried tanh: Sc = sigmoid(2c'(cc)); h'(cc) = So*Tc
                    if idx >= 2:
                        cc = (c - 2) % 4
                        ct = t if c >= 2 else t - 1
                        tc_t = wk.tile([128, CK], BF16, tag=f"c{c}")
                        nc.vector.tensor_scalar(
                            out=tc_t[:, :], in0=sc, scalar1=2.0,
                            scalar2=-1.0, op0=AT.mult, op1=AT.add)
                        so_cc = tt_tiles[cc][:, 2 * CK:3 * CK]
                        if ct == t_steps - 1:
                            # final h -> fp16 hh for the fusion matmul
                            nc.vector.tensor_tensor(
                                out=hh[:, CK * cc:CK * cc + CK],
                                in0=tc_t[:, :], in1=so_cc, op=AT.mult)
                        else:
                            tgt = xh_cur if c >= 2 else xh_prev
                            nc.gpsimd.tensor_tensor(
                                out=tgt[:, CK * cc:CK * cc + CK],
                                in0=tc_t[:, :], in1=so_cc, op=AT.mult)
                    tt_tiles[c] = tt

                # adjacency streaming: one 500KB tile every 3 timesteps
                if t >= 4 and (t - 4) % 3 == 0:
                    k = (t - 4) // 3
                    if k < 16:
                        nc.gpsimd.dma_start(out=a_tiles[k][0:KC, :],
                                            in_=a_mat[KC * k:KC * k + KC, :])

                s_prev = s_cur
                if xh_cur is not None:
                    xh_prev = xh_cur

            # phantom flush: tanh + h for chunks 2,3 of the last timestep
            # (their c' was never carried into an A-op)
            sc_f = wk.tile([128, 2 * CK], BF16, tag="scf", bufs=1)
            nc.scalar.activation(out=sc_f[:, :], in_=s_prev[:, 2 * CK:NPC],
                                 func=AF.Sigmoid, scale=1.0)
            tc_f = wk.tile([128, 2 * CK], BF16, tag="tcf", bufs=1)
            nc.vector.tensor_scalar(
                out=tc_f[:, :], in0=sc_f[:, :], scalar1=2.0, scalar2=-1.0,
                op0=AT.mult, op1=AT.add)
            for j, cc in enumerate((2, 3)):
                nc.vector.tensor_tensor(
                    out=hh[:, CK * cc:CK * cc + CK],
                    in0=tc_f[:, CK * j:CK * j + CK],
                    in1=tt_tiles[cc][:, 2 * CK:3 * CK], op=AT.mult)

            # repeat (device For_i) wraps ONLY the collective-free LSTM;
            # the GNN phase is python-unrolled via gnn_repeat instead
            # (collectives inside a hardware loop wedge the runtime).
            if rep_cm is not None:
                rep_cm.__exit__(None, None, None)
                rep_cm = None
            if _os.environ.get("K_SKIP_GNN"):
                pred0 = wk.tile([1, NPC], F32, tag="pred0", bufs=1)
                nc.vector.tensor_copy(out=pred0[0:1, :], in_=hh[0:1, :])
                nc.sync.dma_start(out=out_d[0:1, :], in_=pred0[0:1, :])
                pl_cm.__exit__(None, None, None)
                return nc
            pl_cm.__exit__(None, None, None)
            pp_cm = tc.tile_pool(name="psg", bufs=2, space="PSUM")
            pp = pp_cm.__enter__()

            # ---- fusion (own NPC nodes) ----
            def mm_halves(psum, pairs, width=NPC):
                # pairs: (lhsT_ap, rhs_tile, rhs_partitions); 500-col slots
                # at GSL strides (psum accumulation groups stay in-bank)
                nh = width // CH
                for c in range(nh):
                    osl = slice(GSL * c, GSL * c + CH)
                    for j, (lt, rtile, pr) in enumerate(pairs):
                        nc.tensor.matmul(
                            out=psum[:, osl], lhsT=lt,
                            rhs=rtile[0:pr, CH * c:CH * c + CH],
                            start=(j == 0), stop=(j == len(pairs) - 1))

            def psum_drain(psum, dst, func, bias=0.0, width=NPC, rows=128):
                # one strided ACT op over the GSL-strided slots
                nh = width // CH
                nc.scalar.activation(
                    out=dst[0:rows, 0:width].rearrange("p (c b) -> p c b", c=nh),
                    in_=psum[0:rows, 0:GSL * nh].rearrange(
                        "p (c b) -> p c b", c=nh)[:, :, 0:CH],
                    func=func, bias=bias, scale=1.0)

            node_t = wk.tile([128, NPC], BF16, tag="node", bufs=1)
            psf = pp.tile([128, 2 * GSL], F32, tag="gp")
            mm_halves(psf, [(w_fz_t[:, :], hh, 128), (w_fs_t[:, :], stl, 128)])
            psum_drain(psf, node_t, AF.Relu, bias=b_fu_t[:, 0:1])

            # ---- single pairwise exchange of fused node features ----
            # (remote_dma would be ~6us vs the collective's 15us fixed
            # overhead, but neither CoreSim nor the axon fake_nrt tunnel
            # resolves the NC topology RDMA needs, so AllGather it is.)
            # All aggregation/dst ordering is [own | peer] per core (host
            # permutes odd cores' adjacency to match); even cores' order is
            # global and only their outputs are read back.
            with tc.tile_pool(name="dram", bufs=1, space="DRAM") as dp:
                # fp8 payload: collective cost scales with the gathered
                # output size (15us fixed + bytes/40GBps), so halving the
                # dtype saves ~6.4us of exposed latency
                node8 = wk.tile([128, NPC], E4, tag="n8", bufs=1)
                nc.vector.tensor_copy(out=node8[:, :], in_=node_t[:, :])
                cc_in = dp.tile([128, NPC], E4, tag="ci")
                cc_out = dp.tile([256, NPC], E4, tag="co")
                nc.sync.dma_start(out=cc_in[:, :], in_=node8[:, :])
                nc.gpsimd.collective_compute(
                    "AllGather", AT.bypass,
                    replica_groups=[[0, 1], [2, 3], [4, 5], [6, 7]],
                    ins=[cc_in.opt()], outs=[cc_out.opt()])

                def trans_chunks(x_t_tile, x_nm, k0):
                    # [128, NPC] feature-major -> 8 chunks [KC, 128] at k0..
                    for k in range(8):
                        trp = pp.tile([128, 128], BF16, tag="tr")
                        nc.tensor.transpose(
                            out=trp[0:KC, :],
                            in_=x_t_tile[:, KC * k:KC * k + KC],
                            identity=ident[:, :])
                        nc.vector.tensor_copy(
                            out=x_nm[0:KC, 128 * (k0 + k):128 * (k0 + k) + 128],
                            in_=trp[0:KC, :])

                def agg_emit(x_nm, psms, ks):
                    for k in ks:
                        for hf in range(2):
                            for dc in range(2):
                                osl = slice(GSL * dc, GSL * dc + CH)
                                nc.tensor.matmul(
                                    out=psms[hf][:, osl],
                                    lhsT=x_nm[0:KC, 128 * k:128 * k + 128],
                                    rhs=a_tiles[k][0:KC,
                                                   NPC * hf + CH * dc:
                                                   NPC * hf + CH * dc + CH],
                                    start=(k == 0), stop=(k == 15))

                for _gr in range(gnn_repeat):
                    # own-half transposes + own-src partial aggregation run
                    # UNDER the collective (they only need local node_t)
                    z_nm = wk.tile([128, 16 * 128], BF16, tag="znm", bufs=1)
                    trans_chunks(node_t, z_nm, 0)
                    psm0 = pp.tile([128, 2 * GSL], F32, tag="gp")
                    psm1 = pp.tile([128, 2 * GSL], F32, tag="gp")
                    agg_emit(z_nm, [psm0, psm1], range(0, 8))

                    # peer features: (block0 + block1) - node8, symmetric
                    zb = wk.tile([128, 2 * NPC], E4, tag="zb", bufs=1)
                    for r, eng in ((0, nc.sync), (1, nc.scalar)):
                        eng.dma_start(
                            out=zb[:, NPC * r:NPC * r + NPC],
                            in_=cc_out[128 * r:128 * r + 128, :])
                    z_peer = wk.tile([128, NPC], BF16, tag="zp", bufs=1)
                    nc.vector.tensor_tensor(
                        out=z_peer[:, :], in0=zb[:, 0:NPC],
                        in1=zb[:, NPC:2 * NPC], op=AT.add)
                    nc.vector.tensor_tensor(
                        out=z_peer[:, :], in0=z_peer[:, :],
                        in1=node8[:, :], op=AT.subtract)
                    trans_chunks(z_peer, z_nm, 8)
                    agg_emit(z_nm, [psm0, psm1], range(8, 16))

                    mean_t = wk.tile([128, N], BF16, tag="m1", bufs=1)
                    h1_t = wk.tile([128, N], BF16, tag="o1", bufs=1)
                    roots = [node_t, z_peer]
                    for hf in range(2):
                        dsl = slice(NPC * hf, NPC * hf + NPC)
                        psum_drain([psm0, psm1][hf], mean_t[:, dsl],
                                   AF.Identity)
                        psh = pp.tile([128, 2 * GSL], F32, tag="gp")
                        mm_halves(psh, [(w_r1_t[:, :], roots[hf], 128),
                                        (w_l1_t[:, :], mean_t[:, dsl], 128)])
                        psum_drain(psh, h1_t[:, dsl], AF.Relu,
                                   bias=b_l1_t[:, 0:1])

                    # ---- SAGE 2: own-half dst only (host stitches the
                    # pair's halves back together) ----
                    h1_nm = wk.tile([128, 16 * 128], BF16, tag="hnm", bufs=1)
                    trans_chunks(h1_t[:, 0:NPC], h1_nm, 0)
                    trans_chunks(h1_t[:, NPC:N], h1_nm, 8)
                    psn0 = pp.tile([128, 2 * GSL], F32, tag="gp")
                    for k in range(16):
                        for dc in range(2):
                            osl = slice(GSL * dc, GSL * dc + CH)
                            nc.tensor.matmul(
                                out=psn0[:, osl],
                                lhsT=h1_nm[0:KC, 128 * k:128 * k + 128],
                                rhs=a_tiles[k][0:KC, CH * dc:CH * dc + CH],
                                start=(k == 0), stop=(k == 15))
                    mean2_t = wk.tile([128, NPC], BF16, tag="m2", bufs=1)
                    h2_t = wk.tile([128, NPC], BF16, tag="o2", bufs=1)
                    psum_drain(psn0, mean2_t[:, :], AF.Identity)
                    psh = pp.tile([128, 2 * GSL], F32, tag="gp")
                    mm_halves(psh, [(w_r2_t[:, :], h1_t[:, 0:NPC], 128),
                                    (w_l2_t[:, :], mean2_t[:, :], 128)])
                    psum_drain(psh, h2_t[:, :], AF.Identity,
                               bias=b_l2_t[:, 0:1])

                # ---- head (own half) ----
                pred = wk.tile([1, NPC], F32, tag="pred", bufs=1)
                pso = pp.tile([1, 2 * GSL], F32, tag="ho", bufs=1)
                for dc in range(2):
                    nc.tensor.matmul(
                        out=pso[0:1, GSL * dc:GSL * dc + CH],
                        lhsT=w_ou_t[:, 0:1],
                        rhs=h2_t[:, CH * dc:CH * dc + CH],
                        start=True, stop=True)
                nc.scalar.activation(
                    out=pred[0:1, :].rearrange("p (c b) -> p c b", c=2),
                    in_=pso[0:1, :].rearrange("p (c b) -> p c b", c=2)
                    [:, :, 0:CH],
                    func=AF.Identity, bias=b_ou_t[0:1, 0:1], scale=1.0)
                nc.sync.dma_start(out=out_d[0:1, :], in_=pred[0:1, :])
            pp_cm.__exit__(None, None, None)

    return nc


def _prep_inputs(inputs, t_steps=T):
    """Host-side preprocessing: per-core input maps."""
    dyn = np.asarray(inputs["dynamic_features"], np.float32)
    sta = np.asarray(inputs["static_features"], np.float32)
    ei = np.asarray(inputs["edge_index"])
    W_ih = np.asarray(inputs["W_ih"], np.float32)
    W_hh = np.asarray(inputs["W_hh"], np.float32)
    b = (np.asarray(inputs["b_ih"], np.float32)
         + np.asarray(inputs["b_hh"], np.float32))
    W_sta = np.asarray(inputs["W_sta"], np.float32)
    b_sta = np.asarray(inputs["b_sta"], np.float32)
    W_fuse = np.asarray(inputs["W_fuse"], np.float32)
    b_fuse = np.asarray(inputs["b_fuse"], np.float32)
    s1_Wl = np.asarray(inputs["sage1_Wl"], np.float32)
    s1_bl = np.asarray(inputs["sage1_bl"], np.float32)
    s1_Wr = np.asarray(inputs["sage1_Wr"], np.float32)
    s2_Wl = np.asarray(inputs["sage2_Wl"], np.float32)
    s2_bl = np.asarray(inputs["sage2_bl"], np.float32)
    s2_Wr = np.asarray(inputs["sage2_Wr"], np.float32)
    W_out = np.asarray(inputs["W_out"], np.float32)
    b_out = np.asarray(inputs["b_out"], np.float32)

    # gate order in psum: [i, f, o, g]; torch order in weights: i,f,g,o
    gsl = [slice(0, H), slice(H, 2 * H), slice(3 * H, 4 * H), slice(2 * H, 3 * H)]
    # w_rec: lhsT [h_in, 4H]; g-gate 2x (tanh via sigmoid of doubled preact)
    w_rec_f32 = np.concatenate(
        [W_hh[gsl[0]].T, W_hh[gsl[1]].T,
         W_hh[gsl[2]].T, 2.0 * W_hh[gsl[3]].T], axis=1)
    # wx9: [9, 4H] = [W_ih^T; bias row], g-gate 2x
    wx9 = np.zeros((9, 4 * H), np.float32)
    for j, s in enumerate(gsl):
        sc = 2.0 if j == 3 else 1.0
        wx9[0:8, H * j:H * j + H] = sc * W_ih[s].T
        wx9[8, H * j:H * j + H] = sc * b[s]

    # fused DoubleRow weight: [W_hh (all rows) | W_x (rows 0..8)]
    w_f = np.zeros((128, 1024), np.float32)
    w_f[:, 0:512] = w_rec_f32
    w_f[0:9, 512:1024] = wx9
    w_f = w_f.astype(E4NP)

    w_sta_t = np.zeros((F_STA + 1, H), np.float32)
    w_sta_t[0:F_STA] = W_sta.T
    w_sta_t[F_STA] = b_sta

    # normalized adjacency (same graph for every batch element)
    src, dst = ei[0].astype(np.int64), ei[1].astype(np.int64)
    cnt = np.bincount(dst, minlength=N).astype(np.float32)
    A = np.zeros((N, N), np.float32)
    np.add.at(A, (src, dst), 1.0)
    A /= np.maximum(cnt, 1.0)[None, :]
    A16 = np.ascontiguousarray(A).astype(BFNP)
    # odd cores aggregate in [own|peer] node order: permute rows+cols
    Pm = np.concatenate([np.arange(NPC, N), np.arange(0, NPC)])
    A16p = np.ascontiguousarray(A[Pm][:, Pm]).astype(BFNP)

    x_bn = dyn.transpose(0, 2, 1, 3).reshape(B * N, dyn.shape[1], F_DYN)
    sta_bn = sta.reshape(B * N, F_STA)

    shared = dict(
        w_f=w_f, w_sta=w_sta_t,
        w_fz=np.ascontiguousarray(W_fuse[:, :H].T.astype(BFNP)),
        w_fs=np.ascontiguousarray(W_fuse[:, H:].T.astype(BFNP)),
        b_fu=b_fuse.reshape(H, 1),
        w_r1=np.ascontiguousarray(s1_Wr.T.astype(BFNP)),
        w_l1=np.ascontiguousarray(s1_Wl.T.astype(BFNP)),
        b_l1=s1_bl.reshape(H, 1),
        w_r2=np.ascontiguousarray(s2_Wr.T.astype(BFNP)),
        w_l2=np.ascontiguousarray(s2_Wl.T.astype(BFNP)),
        b_l2=s2_bl.reshape(H, 1),
        w_ou=np.ascontiguousarray(W_out.T.astype(BFNP)),
        b_ou=b_out.reshape(1, 1),
    )

    in_maps = []
    for core in range(N_CORES):
        rows = slice(NPC * core, NPC * core + NPC)
        xc = x_bn[rows, 0:t_steps, :]                       # [NPC, T, 8]
        xt = xc.transpose(1, 2, 0)                          # [T, 8, NPC]
        arr = np.ones((t_steps, 9, NPC), np.float32)
        arr[:, 0:8, :] = xt
        x8_in = np.ascontiguousarray(
            arr.reshape(t_steps * 9, NPC)).astype(E4NP)

        sta_in = np.ones((F_STA + 1, NPC), np.float32)
        sta_in[0:F_STA] = sta_bn[rows].T

        m = dict(shared)
        m.update(x8c=x8_in, sta_t=sta_in,
                 a_mat=A16 if core % 2 == 0 else A16p)
        in_maps.append(m)
    return in_maps


def kernel(**inputs):
    t_steps = int(np.asarray(inputs["dynamic_features"]).shape[1])
    if t_steps not in _PROG_CACHE:
        nc_new = _build_program(t_steps)
        if not nc_new.is_finalized():
            nc_new.finalize()
        _PROG_CACHE[t_steps] = nc_new
    nc = _PROG_CACHE[t_steps]
    in_maps = _prep_inputs(inputs, t_steps)
    br = run_bass_kernel_spmd(nc, in_maps, list(range(N_CORES)),
                              trace=TRACE, **TRACE_KW)
    kernel.last_result = br
    # each core predicts its own NPC nodes (odd cores' [own|peer] dst
    # ordering puts "own" first, so core c always yields global rows
    # [NPC*c : NPC*c+NPC])
    out = np.concatenate(
        [np.asarray(br.results[c]["out"]).reshape(NPC) for c in range(N_CORES)])
    return out.reshape(B, N).astype(np.float32)


# revision 45
# speedup vs baseline: 1.0038x; 1.0038x over previous
"""Trainium2 Bass kernel for CombinedLSTMWithStatic2Hop.

Model: per-node LSTM over T timesteps + static encoder -> fusion -> 2x SAGEConv
(mean aggregation) -> linear head.

Sharding: B*N = 8000 nodes split into 1000 contiguous nodes per core (8 cores).
Each graph (2000 nodes) spans a core pair; SAGE aggregation uses pairwise
AllGather of node features between the two halves.

LSTM layout: hidden/gate dim on partitions, nodes on the free dim, 4 chunks
of 250 nodes per timestep.  ONE sigmoid ACT op per chunk covers FIVE
256-stride psum slots [i|f|o|g|c]:
  slots 0-3: gates from one DoubleRow fp8 matmul each (g preact 2x scaled
             host-side so sigmoid gives (tanh+1)/2),
  slot 4:    2*c' of the chunk TWO instances back (lag-2 rotation), so
             tanh(c') = 2*sigmoid(2c') - 1 comes out of the same ACT op
             and the ACT engine runs 4 ops/timestep with no separate tanh.
Cell arithmetic runs on DVE scalar_tensor_tensor (InstTensorScalarPtr, 4x
perf mode on packed fp16) with the state kept as S := 2c:
  P  = (Sg * 4) * Si          ; q = (Si * -2) + P   -> q = Si*(4Sg-2)
  t1 = (S * 1) * Sf           ; S' = (t1 * 1) + q   -> S' = 2c'
  c2p: Pool writes (t1+q) into the +2 instance's psum c-slot
  Tc = 2*Sc - 1               ; h' = (Tc * 1) * So  (Pool, fp8 into xh)
x images are compact 9 rows (8 features + ones bias row) at partitions 0-8
for every t; weight rows outside 0..8 of the x-part are zero, so the fused
[W_hh | W_x] DoubleRow weight is group-independent: w_f is [128, 1024] fp8.
"""

import os as _os

import ml_dtypes
import numpy as np

BFNP = np.float16

import concourse.bass as bass
import concourse.tile as tile
from concourse import bacc, mybir
from concourse.bass_utils import run_bass_kernel_spmd
from concourse.masks import make_identity

F32 = mybir.dt.float32
BF16 = mybir.dt.float16  # fp16: same PE rate as bf16, 8x finer mantissa
E4 = mybir.dt.float8e4   # fp8 e4m3: DoubleRow matmul at 2 cols/cycle
E4NP = ml_dtypes.float8_e4m3fn

B, T, N, E = 4, 96, 2000, 16000
F_DYN, F_STA, H = 8, 16, 128
N_CORES = 8
NPC = B * N // N_CORES      # 1000 nodes per core
CH = NPC // 2               # 500 node half (GNN phase)
CK = NPC // 4               # 250 node chunk (LSTM)
SL = 256                    # psum slot stride (f32 elems); 5 slots per chunk
GSL = 512                   # GNN psum slot stride (one bank)

# module-level knobs (test.py may override)
TRACE = False
TRACE_KW = {}

_PROG_CACHE = {}


def _build_program(t_steps=T, repeat=1, gnn_repeat=1):
    nc = bacc.Bacc("TRN2", target_bir_lowering=False, debug=False,
                   num_devices=N_CORES)

    # ---- DRAM I/O ----
    x8c = nc.dram_tensor("x8c", [t_steps * 9, NPC], E4, kind="ExternalInput")
    w_f = nc.dram_tensor("w_f", [128, 1024], E4, kind="ExternalInput")
    w_sta = nc.dram_tensor("w_sta", [F_STA + 1, H], F32, kind="ExternalInput")
    sta_t = nc.dram_tensor("sta_t", [F_STA + 1, NPC], F32, kind="ExternalInput")
    w_fz = nc.dram_tensor("w_fz", [H, H], BF16, kind="ExternalInput")
    w_fs = nc.dram_tensor("w_fs", [H, H], BF16, kind="ExternalInput")
    b_fu = nc.dram_tensor("b_fu", [H, 1], F32, kind="ExternalInput")
    w_r1 = nc.dram_tensor("w_r1", [H, H], BF16, kind="ExternalInput")
    w_l1 = nc.dram_tensor("w_l1", [H, H], BF16, kind="ExternalInput")
    b_l1 = nc.dram_tensor("b_l1", [H, 1], F32, kind="ExternalInput")
    w_r2 = nc.dram_tensor("w_r2", [H, H], BF16, kind="ExternalInput")
    w_l2 = nc.dram_tensor("w_l2", [H, H], BF16, kind="ExternalInput")
    b_l2 = nc.dram_tensor("b_l2", [H, 1], F32, kind="ExternalInput")
    w_ou = nc.dram_tensor("w_ou", [H, 1], BF16, kind="ExternalInput")
    b_ou = nc.dram_tensor("b_ou", [1, 1], F32, kind="ExternalInput")
    a_mat = nc.dram_tensor("a_mat", [N, N], BF16, kind="ExternalInput")
    out_d = nc.dram_tensor("out", [1, NPC], F32, kind="ExternalOutput")

    AT = mybir.AluOpType
    AF = mybir.ActivationFunctionType
    DR = mybir.MatmulPerfMode.DoubleRow

    with tile.TileContext(nc) as tc:
        with (
            tc.tile_pool(name="const", bufs=1) as cp,
            tc.tile_pool(name="xp", bufs=3) as xp,
            tc.tile_pool(name="wk", bufs=2) as wk,
        ):
            # ---- t=0 critical path first on the sync queue: fused weights
            # then the x(0) image ----
            w_f_t = cp.tile([128, 1024], E4, tag="w_f")
            nc.sync.dma_start(out=w_f_t[:, :], in_=w_f[:, :])

            # 3 persistent [h(t) | x(t+1)] buffers rotated manually (pool
            # reuse would trip the race checker on the once-only memset of
            # x-part rows 9..127, which stay zero forever)
            xh_bufs = []
            for i in range(3):
                xb = wk.tile([128, 2 * NPC], E4, tag=f"hb{i}", bufs=1,
                             name=f"xhb{i}")
                nc.vector.memset(xb[:, :] if i == 0 else xb[:, NPC:2 * NPC],
                                 0.0)
                xh_bufs.append(xb)
            xh_first = xh_bufs[0]
            nc.sync.dma_start(out=xh_first[0:9, NPC:2 * NPC],
                              in_=x8c[0:9, :])

            def cload(dram, shape, tag, dt=F32, eng=nc.gpsimd):
                tl = cp.tile(shape, dt, tag=tag)
                eng.dma_start(out=tl[:, :], in_=dram[:, :])
                return tl

            w_sta_tt = cload(w_sta, [F_STA + 1, H], "w_sta", eng=nc.sync)
            sta_tt = cload(sta_t, [F_STA + 1, NPC], "sta_t", eng=nc.sync)
            w_fz_t = cload(w_fz, [H, H], "w_fz", BF16)
            w_fs_t = cload(w_fs, [H, H], "w_fs", BF16)
            b_fu_t = cload(b_fu, [H, 1], "b_fu")
            w_r1_t = cload(w_r1, [H, H], "w_r1", BF16)
            w_l1_t = cload(w_l1, [H, H], "w_l1", BF16)
            b_l1_t = cload(b_l1, [H, 1], "b_l1")
            w_r2_t = cload(w_r2, [H, H], "w_r2", BF16)
            w_l2_t = cload(w_l2, [H, H], "w_l2", BF16)
            b_l2_t = cload(b_l2, [H, 1], "b_l2")
            w_ou_t = cload(w_ou, [H, 1], "w_ou", BF16)
            b_ou_t = cload(b_ou, [1, 1], "b_ou")

            ident = cp.tile([128, 128], BF16, tag="ident")
            make_identity(nc, ident[:, :])

            # adjacency tiles: allocated now, DMAs spread through the LSTM
            # loop on the gpsimd queue (Pool has slack between cell ops)
            a_tiles = []
            KC = N // 16  # 125-row src chunks over the FULL graph adjacency
            for k in range(16):
                a_tiles.append(cp.tile([KC, N], BF16, tag=f"a{k}",
                                       name=f"a{k}"))

            w_f_r = w_f_t[:, :].rearrange("p (s m) -> p s m", s=2)

            # ---- LSTM ----
            pl_cm = tc.tile_pool(name="psl", bufs=2, space="PSUM")
            pl = pl_cm.__enter__()

            # static encoder first: fills the w_f-load gap; its psum tile
            # aliases ps[1]'s buffer (read drained before instance idx=1)
            stl = wk.tile([128, NPC], BF16, tag="stl", bufs=1)
            pss = pl.tile([128, 5 * SL], F32, tag="gates", name="pss")
            for dc in range(4):
                nc.tensor.matmul(
                    out=pss[:, SL * dc:SL * dc + CK],
                    lhsT=w_sta_tt[0:17, :],
                    rhs=sta_tt[0:17, CK * dc:CK * dc + CK],
                    start=True, stop=True)
            nc.scalar.activation(
                out=stl[:, :].rearrange("p (c b) -> p c b", c=4),
                in_=pss[:, 0:4 * SL].rearrange("p (c b) -> p c b", c=4)
                [:, :, 0:CK],
                func=AF.Relu, scale=1.0)

            ps = [pl.tile([128, 5 * SL], F32, tag="gates", name="gates0"),
                  pl.tile([128, 5 * SL], F32, tag="gates", name="gates1")]
            # c-slot read by A(0)/A(1) before any c2p lands
            for pst in ps:
                nc.vector.memset(pst[:, 4 * SL:4 * SL + CK], 0.0)

            # final LSTM hidden state (fp16) for the fusion matmul
            hh = wk.tile([128, NPC], BF16, tag="hh16", bufs=1)

            rep_cm = tc.For_i(0, repeat, 1) if repeat > 1 else None
            if rep_cm is not None:
                rep_cm.__enter__()

            s_prev = None
            xh_prev = xh_first
            tt_tiles = {}
            for t in range(t_steps):
                s_cur = wk.tile([128, NPC], BF16, tag="s", bufs=2)
                if t + 1 < t_steps:
                    xh_cur = xh_bufs[(t + 1) % 3]
                    nc.sync.dma_start(
                        out=xh_cur[0:9, NPC:2 * NPC],
                        in_=x8c[9 * (t + 1):9 * (t + 1) + 9, :])
                else:
                    xh_cur = None
                xh_rhs = xh_prev[:, :].rearrange("p (s n) -> p s n", s=2)

                for c in range(4):
                    idx = 4 * t + c
                    pst = ps[idx % 2]
                    csl = slice(CK * c, CK * c + CK)

                    # gates: one DoubleRow mm per slot (W_hh@h + W_x@x)
                    for gi in range(4):
                        nc.tensor.matmul(
                            out=pst[:, SL * gi:SL * gi + CK],
                            lhsT=w_f_r[:, :, H * gi:H * gi + H],
                            rhs=xh_rhs[:, :, csl],
                            start=True, stop=True, perf_mode=DR)

                    # ONE sigmoid over 5 slots: [Si|Sf|So|Sg|Sc]
                    tt = wk.tile([128, 5 * CK], BF16, tag=f"T{c}",
                                 name=f"tt{c}_{t}")
                    nc.scalar.activation(
                        out=tt[:, :].rearrange("p (g b) -> p g b", g=5),
                        in_=pst[:, :].rearrange("p (s b) -> p s b", s=5)
                        [:, :, 0:CK],
                        func=AF.Sigmoid, scale=1.0)
                    si = tt[:, 0:CK]
                    sf = tt[:, CK:2 * CK]
                    sg = tt[:, 3 * CK:4 * CK]
                    sc = tt[:, 4 * CK:5 * CK]

                    # cell math, S' = Sf*S + Si*(4Sg-2), on ts(4x)/TT(2x)
                    # ops (scalar_tensor_tensor has NO dve perf modes)
                    tg_t = wk.tile([128, CK], BF16, tag=f"p{c}")
                    nc.vector.tensor_scalar(
                        out=tg_t[:, :], in0=sg, scalar1=4.0, scalar2=-2.0,
                        op0=AT.mult, op1=AT.add)
                    t2_t = wk.tile([128, CK], BF16, tag=f"q{c}")
                    nc.vector.tensor_tensor(
                        out=t2_t[:, :], in0=si, in1=tg_t[:, :], op=AT.mult)
                    if t > 0:
                        # t1 on Pool (GPSIMD cannot access PSUM, SBUF ok)
                        t1_t = wk.tile([128, CK], BF16, tag=f"u{c}")
                        nc.gpsimd.tensor_tensor(
                            out=t1_t[:, :], in0=s_prev[:, csl], in1=sf,
                            op=AT.mult)
                        nc.vector.tensor_tensor(
                            out=s_cur[:, csl], in0=t1_t[:, :],
                            in1=t2_t[:, :], op=AT.add)
                    else:
                        nc.vector.tensor_copy(out=s_cur[:, csl],
                                              in_=t2_t[:, :])
                    if not (t == t_steps - 1 and c >= 2):
                        # c-slot psum write via PE identity matmul: the
                        # only engine with cheap PSUM writes (PE is idle)
                        nc.tensor.matmul(
                            out=pst[:, 4 * SL:4 * SL + CK],
                            lhsT=ident[:, :], rhs=s_cur[:, csl],
                            start=True, stop=True)

                    # carried tanh: Sc = sigmoid(2c'(cc)); h'(cc) = So*Tc
                    if idx >= 2:
                        cc = (c - 2) % 4
                        ct = t if c >= 2 else t - 1
                        tc_t = wk.tile([128, CK], BF16, tag=f"c{c}")
                        nc.vector.tensor_scalar(
                            out=tc_t[:, :], in0=sc, scalar1=2.0,
                            scalar2=-1.0, op0=AT.mult, op1=AT.add)
                        so_cc = tt_tiles[cc][:, 2 * CK:3 * CK]
                        if ct == t_steps - 1:
                            # final h -> fp16 hh for the fusion matmul
                            nc.vector.tensor_tensor(
                                out=hh[:, CK * cc:CK * cc + CK],
                                in0=tc_t[:, :], in1=so_cc, op=AT.mult)
                        else:
                            tgt = xh_cur if c >= 2 else xh_prev
                            nc.gpsimd.tensor_tensor(
                                out=tgt[:, CK * cc:CK * cc + CK],
                                in0=tc_t[:, :], in1=so_cc, op=AT.mult)
                    tt_tiles[c] = tt

                # adjacency streaming: one 500KB tile every 3 timesteps
                if t >= 4 and (t - 4) % 3 == 0:
                    k = (t - 4) // 3
                    if k < 16:
                        nc.gpsimd.dma_start(out=a_tiles[k][0:KC, :],
                                            in_=a_mat[KC * k:KC * k + KC, :])

                s_prev = s_cur
                if xh_cur is not None:
                    xh_prev = xh_cur

            # phantom flush: tanh + h for chunks 2,3 of the last timestep
            # (their c' was never carried into an A-op)
            sc_f = wk.tile([128, 2 * CK], BF16, tag="scf", bufs=1)
            nc.scalar.activation(out=sc_f[:, :], in_=s_prev[:, 2 * CK:NPC],
                                 func=AF.Sigmoid, scale=1.0)
            tc_f = wk.tile([128, 2 * CK], BF16, tag="tcf", bufs=1)
            nc.vector.tensor_scalar(
                out=tc_f[:, :], in0=sc_f[:, :], scalar1=2.0, scalar2=-1.0,
                op0=AT.mult, op1=AT.add)
            for j, cc in enumerate((2, 3)):
                nc.vector.tensor_tensor(
                    out=hh[:, CK * cc:CK * cc + CK],
                    in0=tc_f[:, CK * j:CK * j + CK],
                    in1=tt_tiles[cc][:, 2 * CK:3 * CK], op=AT.mult)

            # repeat (device For_i) wraps ONLY the collective-free LSTM;
            # the GNN phase is python-unrolled via gnn_repeat instead
            # (collectives inside a hardware loop wedge the runtime).
            if rep_cm is not None:
                rep_cm.__exit__(None, None, None)
                rep_cm = None
            if _os.environ.get("K_SKIP_GNN"):
                pred0 = wk.tile([1, NPC], F32, tag="pred0", bufs=1)
                nc.vector.tensor_copy(out=pred0[0:1, :], in_=hh[0:1, :])
                nc.sync.dma_start(out=out_d[0:1, :], in_=pred0[0:1, :])
                pl_cm.__exit__(None, None, None)
                return nc
            pl_cm.__exit__(None, None, None)
            pp_cm = tc.tile_pool(name="psg", bufs=2, space="PSUM")
            pp = pp_cm.__enter__()

            # ---- fusion (own NPC nodes) ----
            def mm_halves(psum, pairs, width=NPC):
                # pairs: (lhsT_ap, rhs_tile, rhs_partitions); 500-col slots
                # at GSL strides (psum accumulation groups stay in-bank)
                nh = width // CH
                for c in range(nh):
                    osl = slice(GSL * c, GSL * c + CH)
                    for j, (lt, rtile, pr) in enumerate(pairs):
                        nc.tensor.matmul(
                            out=psum[:, osl], lhsT=lt,
                            rhs=rtile[0:pr, CH * c:CH * c + CH],
                            start=(j == 0), stop=(j == len(pairs) - 1))

            def psum_drain(psum, dst, func, bias=0.0, width=NPC, rows=128):
                # one strided ACT op over the GSL-strided slots
                nh = width // CH
                nc.scalar.activation(
                    out=dst[0:rows, 0:width].rearrange("p (c b) -> p c b", c=nh),
                    in_=psum[0:rows, 0:GSL * nh].rearrange(
                        "p (c b) -> p c b", c=nh)[:, :, 0:CH],
                    func=func, bias=bias, scale=1.0)

            node_t = wk.tile([128, NPC], BF16, tag="node", bufs=1)
            psf = pp.tile([128, 2 * GSL], F32, tag="gp")
            mm_halves(psf, [(w_fz_t[:, :], hh, 128), (w_fs_t[:, :], stl, 128)])
            psum_drain(psf, node_t, AF.Relu, bias=b_fu_t[:, 0:1])

            # ---- single pairwise exchange of fused node features ----
            # (remote_dma would be ~6us vs the collective's 15us fixed
            # overhead, but neither CoreSim nor the axon fake_nrt tunnel
            # resolves the NC topology RDMA needs, so AllGather it is.)
            # All aggregation/dst ordering is [own | peer] per core (host
            # permutes odd cores' adjacency to match); even cores' order is
            # global and only their outputs are read back.
            with tc.tile_pool(name="dram", bufs=1, space="DRAM") as dp:
                # fp8 payload: collective cost scales with the gathered
                # output size (15us fixed + bytes/40GBps), so halving the
                # dtype saves ~6.4us of exposed latency
                node8 = wk.tile([128, NPC], E4, tag="n8", bufs=1)
                nc.vector.tensor_copy(out=node8[:, :], in_=node_t[:, :])
                cc_in = dp.tile([128, NPC], E4, tag="ci")
                cc_out = dp.tile([256, NPC], E4, tag="co")
                nc.sync.dma_start(out=cc_in[:, :], in_=node8[:, :])
                nc.gpsimd.collective_compute(
                    "AllGather", AT.bypass,
                    replica_groups=[[0, 1], [2, 3], [4, 5], [6, 7]],
                    ins=[cc_in.opt()], outs=[cc_out.opt()])

                def trans_chunks(x_t_tile, x_nm, k0):
                    # [128, NPC] feature-major -> 8 chunks [KC, 128] at k0..
                    for k in range(8):
                        trp = pp.tile([128, 128], BF16, tag="tr")
                        nc.tensor.transpose(
                            out=trp[0:KC, :],
                            in_=x_t_tile[:, KC * k:KC * k + KC],
                            identity=ident[:, :])
                        nc.vector.tensor_copy(
                            out=x_nm[0:KC, 128 * (k0 + k):128 * (k0 + k) + 128],
                            in_=trp[0:KC, :])

                def agg_emit(x_nm, psms, ks):
                    for k in ks:
                        for hf in range(2):
                            for dc in range(2):
                                osl = slice(GSL * dc, GSL * dc + CH)
                                nc.tensor.matmul(
                                    out=psms[hf][:, osl],
                                    lhsT=x_nm[0:KC, 128 * k:128 * k + 128],
                                    rhs=a_tiles[k][0:KC,
                                                   NPC * hf + CH * dc:
                                                   NPC * hf + CH * dc + CH],
                                    start=(k == 0), stop=(k == 15))

                for _gr in range(gnn_repeat):
                    # own-half transposes + own-src partial aggregation run
                    # UNDER the collective (they only need local node_t)
                    z_nm = wk.tile([128, 16 * 128], BF16, tag="znm", bufs=1)
                    trans_chunks(node_t, z_nm, 0)
                    psm0 = pp.tile([128, 2 * GSL], F32, tag="gp")
                    psm1 = pp.tile([128, 2 * GSL], F32, tag="gp")
                    agg_emit(z_nm, [psm0, psm1], range(0, 8))

                    # peer features: (block0 + block1) - node8, symmetric
                    zb = wk.tile([128, 2 * NPC], E4, tag="zb", bufs=1)
                    for r, eng in ((0, nc.sync), (1, nc.scalar)):
                        eng.dma_start(
                            out=zb[:, NPC * r:NPC * r + NPC],
                            in_=cc_out[128 * r:128 * r + 128, :])
                    # reconstruct + transpose the peer features per 125-col
                    # chunk so the first transpose starts ~1.5us earlier
                    # than a full-width reconstruct would allow
                    z_peer = wk.tile([128, NPC], BF16, tag="zp", bufs=1)
                    for k in range(8):
                        zs = slice(KC * k, KC * k + KC)
                        zs1 = slice(NPC + KC * k, NPC + KC * k + KC)
                        nc.vector.tensor_tensor(
                            out=z_peer[:, zs], in0=zb[:, zs],
                            in1=zb[:, zs1], op=AT.add)
                        nc.vector.tensor_tensor(
                            out=z_peer[:, zs], in0=z_peer[:, zs],
                            in1=node8[:, zs], op=AT.subtract)
                        trp = pp.tile([128, 128], BF16, tag="tr")
                        nc.tensor.transpose(
                            out=trp[0:KC, :], in_=z_peer[:, zs],
                            identity=ident[:, :])
                        nc.vector.tensor_copy(
                            out=z_nm[0:KC, 128 * (8 + k):128 * (8 + k) + 128],
                            in_=trp[0:KC, :])
                    agg_emit(z_nm, [psm0, psm1], range(8, 16))

                    mean_t = wk.tile([128, N], BF16, tag="m1", bufs=1)
                    h1_t = wk.tile([128, N], BF16, tag="o1", bufs=1)
                    roots = [node_t, z_peer]
                    for hf in range(2):
                        dsl = slice(NPC * hf, NPC * hf + NPC)
                        psum_drain([psm0, psm1][hf], mean_t[:, dsl],
                                   AF.Identity)
                        psh = pp.tile([128, 2 * GSL], F32, tag="gp")
                        mm_halves(psh, [(w_r1_t[:, :], roots[hf], 128),
                                        (w_l1_t[:, :], mean_t[:, dsl], 128)])
                        psum_drain(psh, h1_t[:, dsl], AF.Relu,
                                   bias=b_l1_t[:, 0:1])

                    # ---- SAGE 2: own-half dst only (host stitches the
                    # pair's halves back together) ----
                    h1_nm = wk.tile([128, 16 * 128], BF16, tag="hnm", bufs=1)
                    trans_chunks(h1_t[:, 0:NPC], h1_nm, 0)
                    trans_chunks(h1_t[:, NPC:N], h1_nm, 8)
                    psn0 = pp.tile([128, 2 * GSL], F32, tag="gp")
                    for k in range(16):
                        for dc in range(2):
                            osl = slice(GSL * dc, GSL * dc + CH)
                            nc.tensor.matmul(
                                out=psn0[:, osl],
                                lhsT=h1_nm[0:KC, 128 * k:128 * k + 128],
                                rhs=a_tiles[k][0:KC, CH * dc:CH * dc + CH],
                                start=(k == 0), stop=(k == 15))
                    mean2_t = wk.tile([128, NPC], BF16, tag="m2", bufs=1)
                    h2_t = wk.tile([128, NPC], BF16, tag="o2", bufs=1)
                    psum_drain(psn0, mean2_t[:, :], AF.Identity)
                    psh = pp.tile([128, 2 * GSL], F32, tag="gp")
                    mm_halves(psh, [(w_r2_t[:, :], h1_t[:, 0:NPC], 128),
                                    (w_l2_t[:, :], mean2_t[:, :], 128)])
                    psum_drain(psh, h2_t[:, :], AF.Identity,
                               bias=b_l2_t[:, 0:1])

                # ---- head (own half) ----
                pred = wk.tile([1, NPC], F32, tag="pred", bufs=1)
                pso = pp.tile([1, 2 * GSL], F32, tag="ho", bufs=1)
                for dc in range(2):
                    nc.tensor.matmul(
                        out=pso[0:1, GSL * dc:GSL * dc + CH],
                        lhsT=w_ou_t[:, 0:1],
                        rhs=h2_t[:, CH * dc:CH * dc + CH],
                        start=True, stop=True)
                nc.scalar.activation(
                    out=pred[0:1, :].rearrange("p (c b) -> p c b", c=2),
                    in_=pso[0:1, :].rearrange("p (c b) -> p c b", c=2)
                    [:, :, 0:CH],
                    func=AF.Identity, bias=b_ou_t[0:1, 0:1], scale=1.0)
                nc.sync.dma_start(out=out_d[0:1, :], in_=pred[0:1, :])
            pp_cm.__exit__(None, None, None)

    return nc


def _prep_inputs(inputs, t_steps=T):
    """Host-side preprocessing: per-core input maps."""
    dyn = np.asarray(inputs["dynamic_features"], np.float32)
    sta = np.asarray(inputs["static_features"], np.float32)
    ei = np.asarray(inputs["edge_index"])
    W_ih = np.asarray(inputs["W_ih"], np.float32)
    W_hh = np.asarray(inputs["W_hh"], np.float32)
    b = (np.asarray(inputs["b_ih"], np.float32)
         + np.asarray(inputs["b_hh"], np.float32))
    W_sta = np.asarray(inputs["W_sta"], np.float32)
    b_sta = np.asarray(inputs["b_sta"], np.float32)
    W_fuse = np.asarray(inputs["W_fuse"], np.float32)
    b_fuse = np.asarray(inputs["b_fuse"], np.float32)
    s1_Wl = np.asarray(inputs["sage1_Wl"], np.float32)
    s1_bl = np.asarray(inputs["sage1_bl"], np.float32)
    s1_Wr = np.asarray(inputs["sage1_Wr"], np.float32)
    s2_Wl = np.asarray(inputs["sage2_Wl"], np.float32)
    s2_bl = np.asarray(inputs["sage2_bl"], np.float32)
    s2_Wr = np.asarray(inputs["sage2_Wr"], np.float32)
    W_out = np.asarray(inputs["W_out"], np.float32)
    b_out = np.asarray(inputs["b_out"], np.float32)

    # gate order in psum: [i, f, o, g]; torch order in weights: i,f,g,o
    gsl = [slice(0, H), slice(H, 2 * H), slice(3 * H, 4 * H), slice(2 * H, 3 * H)]
    # w_rec: lhsT [h_in, 4H]; g-gate 2x (tanh via sigmoid of doubled preact)
    w_rec_f32 = np.concatenate(
        [W_hh[gsl[0]].T, W_hh[gsl[1]].T,
         W_hh[gsl[2]].T, 2.0 * W_hh[gsl[3]].T], axis=1)
    # wx9: [9, 4H] = [W_ih^T; bias row], g-gate 2x
    wx9 = np.zeros((9, 4 * H), np.float32)
    for j, s in enumerate(gsl):
        sc = 2.0 if j == 3 else 1.0
        wx9[0:8, H * j:H * j + H] = sc * W_ih[s].T
        wx9[8, H * j:H * j + H] = sc * b[s]

    # fused DoubleRow weight: [W_hh (all rows) | W_x (rows 0..8)]
    w_f = np.zeros((128, 1024), np.float32)
    w_f[:, 0:512] = w_rec_f32
    w_f[0:9, 512:1024] = wx9
    w_f = w_f.astype(E4NP)

    w_sta_t = np.zeros((F_STA + 1, H), np.float32)
    w_sta_t[0:F_STA] = W_sta.T
    w_sta_t[F_STA] = b_sta

    # normalized adjacency (same graph for every batch element)
    src, dst = ei[0].astype(np.int64), ei[1].astype(np.int64)
    cnt = np.bincount(dst, minlength=N).astype(np.float32)
    A = np.zeros((N, N), np.float32)
    np.add.at(A, (src, dst), 1.0)
    A /= np.maximum(cnt, 1.0)[None, :]
    A16 = np.ascontiguousarray(A).astype(BFNP)
    # odd cores aggregate in [own|peer] node order: permute rows+cols
    Pm = np.concatenate([np.arange(NPC, N), np.arange(0, NPC)])
    A16p = np.ascontiguousarray(A[Pm][:, Pm]).astype(BFNP)

    x_bn = dyn.transpose(0, 2, 1, 3).reshape(B * N, dyn.shape[1], F_DYN)
    sta_bn = sta.reshape(B * N, F_STA)

    shared = dict(
        w_f=w_f, w_sta=w_sta_t,
        w_fz=np.ascontiguousarray(W_fuse[:, :H].T.astype(BFNP)),
        w_fs=np.ascontiguousarray(W_fuse[:, H:].T.astype(BFNP)),
        b_fu=b_fuse.reshape(H, 1),
        w_r1=np.ascontiguousarray(s1_Wr.T.astype(BFNP)),
        w_l1=np.ascontiguousarray(s1_Wl.T.astype(BFNP)),
        b_l1=s1_bl.reshape(H, 1),
        w_r2=np.ascontiguousarray(s2_Wr.T.astype(BFNP)),
        w_l2=np.ascontiguousarray(s2_Wl.T.astype(BFNP)),
        b_l2=s2_bl.reshape(H, 1),
        w_ou=np.ascontiguousarray(W_out.T.astype(BFNP)),
        b_ou=b_out.reshape(1, 1),
    )

    in_maps = []
    for core in range(N_CORES):
        rows = slice(NPC * core, NPC * core + NPC)
        xc = x_bn[rows, 0:t_steps, :]                       # [NPC, T, 8]
        xt = xc.transpose(1, 2, 0)                          # [T, 8, NPC]
        arr = np.ones((t_steps, 9, NPC), np.float32)
        arr[:, 0:8, :] = xt
        x8_in = np.ascontiguousarray(
            arr.reshape(t_steps * 9, NPC)).astype(E4NP)

        sta_in = np.ones((F_STA + 1, NPC), np.float32)
        sta_in[0:F_STA] = sta_bn[rows].T

        m = dict(shared)
        m.update(x8c=x8_in, sta_t=sta_in,
                 a_mat=A16 if core % 2 == 0 else A16p)
        in_maps.append(m)
    return in_maps


def kernel(**inputs):
    t_steps = int(np.asarray(inputs["dynamic_features"]).shape[1])
    if t_steps not in _PROG_CACHE:
        nc_new = _build_program(t_steps)
        if not nc_new.is_finalized():
            nc_new.finalize()
        _PROG_CACHE[t_steps] = nc_new
    nc = _PROG_CACHE[t_steps]
    in_maps = _prep_inputs(inputs, t_steps)
    br = run_bass_kernel_spmd(nc, in_maps, list(range(N_CORES)),
                              trace=TRACE, **TRACE_KW)
    kernel.last_result = br
    # each core predicts its own NPC nodes (odd cores' [own|peer] dst
    # ordering puts "own" first, so core c always yields global rows
    # [NPC*c : NPC*c+NPC])
    out = np.concatenate(
        [np.asarray(br.results[c]["out"]).reshape(NPC) for c in range(N_CORES)])
    return out.reshape(B, N).astype(np.float32)
